# revision 25
# baseline (speedup 1.0000x reference)
"""MoE gate kernel for Trainium2 (8 NeuronCores, SPMD).

Computes, for hidden_states [4, 4096, 4096] f32 and gate_weight [8, 4096] f32:
    logits = hidden @ gate_weight.T          # [tokens, 8]
    p      = softmax(logits)                 # [tokens, 8]
    topk_w, topk_i = top_k(p, 2); topk_w /= topk_w.sum(-1, keepdims=True)

Sharding: data-parallel over tokens (B*S = 16384 -> 2048 tokens/core); the
tiny gate weight is replicated, pre-arranged on host.

Layout: hidden_states is transposed on the host during sharding (xT[h, t],
one [4096, 2048] block per core) and split into a round-to-nearest bf16
hi/lo pair, xh + xl (same 4 bytes/element of DMA traffic as f32, residual
~2^-18).  The gate weight is likewise split (host):

    w_hl[p, hc, 0:8]  = bf16(W)[e, 32p+hc],  w_hl[.., 8:16] = bf16(W - Wh)
    w_h0[p, hc, 0:8]  = bf16(W)[e, 32p+hc],  w_h0[.., 8:16] = 0

so  logits = Xh@Wh + Xh@Wl + Xl@Wh  (the dropped Xl@Wl term is O(2^-18))
computes as TWO bf16 matmuls per h-chunk:  Xh against [Wh|Wl] and Xl
against [Wh|0], accumulating in fp32 PSUM [t, 16]; the hi/lo halves fold
with one DVE add at the end.  Error measured ~1e-5 relative on logits --
fp32-class for the top-2 selection.

Why this shape (all measured on this container's TRN2 cores):
  * DMA: 16KB-run descriptors reach ~380 GB/s/core, 512B runs only ~225
    GB/s.  The transposed h-major layout gives 4KB runs (2048 bf16 per
    row) -> ~88us/rep DMA floor.  Every on-device transpose alternative
    (PE transpose, DMA-XBAR (16-bit only), strided gathers) is slower.
  * PE: fp32 matmuls cost ~420ns each at M=128 (the self-loading fp32
    weight path is slow and serial); bf16 stationary loads use FWL
    (4 XBUSes) and the moving stream runs 1 cyc/row -> the whole gate
    matmul drops under the DMA floor.
  * PSUM: accumulation groups are per zero-region, and chained matmuls
    into one region stall on the read-modify-write; pa/pb pair
    interleaving keeps consecutive matmuls on distinct banks.

Top-2 + renorm uses the DVE max/max_index sort instructions; the
renormalized weights reduce to w1 = 1/(1+exp(m2-m1)), w2 = 1-w1 (the
full-softmax denominator cancels in the reference's top-k renorm).

Walrus's TPB instruction encodings carry a single sync-wait slot, so a
post-pass hoists surplus Tile-generated waits onto same-engine
EventSemaphore prefix instructions (semantics-preserving).
"""

import numpy as np

H = 4096            # hidden size
E = 8               # experts
EW = 2 * E          # psum cols: [hi-expert | lo-expert]
P = 128             # SBUF partitions
HC = H // P         # 32 h-chunks of 128 (chunk c: h = 32p + c, see pack_w)
NR = 4              # DMA ranges per rep (8 h-chunks each)
HC_R = HC // NR     # 8 h-chunks per range
T_TILE = 128        # tokens per PSUM block
N_CORES = 8
TOKENS_TOTAL = 4 * 4096
TOKENS_PER_CORE = TOKENS_TOTAL // N_CORES   # 2048
N_SLOTS = TOKENS_PER_CORE // T_TILE         # 16 blocks of 128 tokens


def _emit_body(nc, mybir, pools, w_hl, w_h0, xh_r, xl_r, wq, iq):
    xpool, cpool, spool, psum_pool = pools
    acc = cpool.acc

    for r in range(NR):
        xh = xpool.tile(
            [P, HC_R, TOKENS_PER_CORE], mybir.dt.bfloat16, tag="xh"
        )
        nc.sync.dma_start(xh[:], xh_r[r])
        xl = xpool.tile(
            [P, HC_R, TOKENS_PER_CORE], mybir.dt.bfloat16, tag="xl"
        )
        nc.sync.dma_start(xl[:], xl_r[r])
        # two PSUM banks (pa/pb) interleave so consecutive matmuls hit
        # distinct accumulation regions; each range's 8-chunk partial
        # folds into the SBUF accumulator on the DVE
        for pr in range(N_SLOTS // 2):
            b0, b1 = 2 * pr, 2 * pr + 1
            pa = psum_pool.tile([T_TILE, EW], mybir.dt.float32, tag="pa")
            pb = psum_pool.tile([T_TILE, EW], mybir.dt.float32, tag="pb")
            for c8 in range(HC_R):
                c = r * HC_R + c8
                s0 = c8 == 0
                s1 = c8 == HC_R - 1
                t0 = b0 * T_TILE
                t1 = b1 * T_TILE
                nc.tensor.matmul(
                    pa[:], xh[:, c8, t0 : t0 + T_TILE], w_hl[:, c, :],
                    start=s0, stop=False,
                )
                nc.tensor.matmul(
                    pb[:], xh[:, c8, t1 : t1 + T_TILE], w_hl[:, c, :],
                    start=s0, stop=False,
                )
                nc.tensor.matmul(
                    pa[:], xl[:, c8, t0 : t0 + T_TILE], w_h0[:, c, :],
                    start=False, stop=s1,
                )
                nc.tensor.matmul(
                    pb[:], xl[:, c8, t1 : t1 + T_TILE], w_h0[:, c, :],
                    start=False, stop=s1,
                )
            if r == 0:
                nc.vector.tensor_copy(acc[:, b0], pa[:])
                nc.vector.tensor_copy(acc[:, b1], pb[:])
            else:
                nc.vector.tensor_add(acc[:, b0], acc[:, b0], pa[:])
                nc.vector.tensor_add(acc[:, b1], acc[:, b1], pb[:])

    # fold hi/lo expert halves: logits[t, slot, e] = acc[.., e] + acc[.., 8+e]
    lg = spool.tile([P, N_SLOTS, E], mybir.dt.float32, tag="lg")
    nc.vector.tensor_add(lg[:], acc[:, :, 0:E], acc[:, :, E:EW])

    for tt in range(N_SLOTS):
        nc.vector.max(out=cpool.sorted_w[:, tt], in_=lg[:, tt])
        nc.vector.max_index(
            out=cpool.idx_w[:, tt], in_max=cpool.sorted_w[:, tt],
            in_values=lg[:, tt],
        )

    sorted_w, idx_w = cpool.sorted_w, cpool.idx_w
    # Batched renormalization over all blocks: w1 = 1/(1+e^(m2-m1)),
    # w2 = e^(m2-m1)/(1+e^(m2-m1)).
    m1 = sorted_w[:, :, 0]
    m2 = sorted_w[:, :, 1]
    d = cpool.tile([P, N_SLOTS], mybir.dt.float32)
    nc.vector.tensor_sub(d[:], m2, m1)
    t = cpool.tile([P, N_SLOTS], mybir.dt.float32)
    nc.scalar.activation(t[:], d[:], mybir.ActivationFunctionType.Exp)
    denom = cpool.tile([P, N_SLOTS], mybir.dt.float32)
    nc.vector.tensor_scalar_add(denom[:], t[:], 1.0)
    r_ = cpool.tile([P, N_SLOTS], mybir.dt.float32)
    nc.vector.reciprocal(r_[:], denom[:])

    wout = cpool.tile([P, N_SLOTS, 2], mybir.dt.float32)
    nc.vector.tensor_copy(wout[:, :, 0], r_[:])
    nc.vector.tensor_mul(wout[:, :, 1], t[:], r_[:])
    iout = cpool.tile([P, N_SLOTS, 2], mybir.dt.uint32)
    nc.vector.tensor_copy(iout[:], idx_w[:, :, 0:2])

    nc.gpsimd.dma_start(wq[:], wout[:])
    nc.gpsimd.dma_start(iq[:], iout[:])


def _legalize_sync_waits(nc, mybir):
    """Split surplus sync waits onto EventSemaphore prefix instructions.

    Walrus's TPB instruction structs have a single `events` wait slot and
    reject instructions with more sync waits.  The same engine sequencer
    executes an EventSemaphore (CTRL_ES) wait-only instruction in program
    order, so hoisting all but one wait onto ES prefixes is
    semantics-preserving.
    """
    limit = 1
    n = 0
    for bb in nc.main_func.blocks:
        out, changed = [], False
        for ins in bb.instructions:
            si = ins.sync_info
            if si is not None and len(si.on_wait) > limit:
                waits = list(si.on_wait)
                for w in waits[:-limit]:
                    es = mybir.InstEventSemaphore(
                        name=f"ESleg-{n}", engine=ins.engine, ins=[], outs=[],
                        sync_info=mybir.SyncInfo(on_wait=[w], on_update=[]),
                    )
                    out.append(es)
                    n += 1
                ins.sync_info = mybir.SyncInfo(
                    on_wait=waits[-limit:], on_update=list(si.on_update)
                )
                changed = True
            out.append(ins)
        if changed:
            bb.instructions = out
    return n


def build_program(reps: int = 1, legalize: bool = True, xpool_bufs: int = 2):
    import concourse.bass as bass
    import concourse.mybir as mybir
    from concourse.tile import TileContext

    nc = bass.Bass("TRN2", debug=False)
    xh_d = nc.declare_dram_parameter(
        "xh", [H, TOKENS_PER_CORE], mybir.dt.bfloat16, isOutput=False
    )
    xl_d = nc.declare_dram_parameter(
        "xl", [H, TOKENS_PER_CORE], mybir.dt.bfloat16, isOutput=False
    )
    whl_d = nc.declare_dram_parameter(
        "whl", [P, HC, EW], mybir.dt.bfloat16, isOutput=False
    )
    wh0_d = nc.declare_dram_parameter(
        "wh0", [P, HC, EW], mybir.dt.bfloat16, isOutput=False
    )
    wq = nc.declare_dram_parameter(
        "wq", [P, N_SLOTS, 2], mybir.dt.float32, isOutput=True
    )
    iq = nc.declare_dram_parameter(
        "iq", [P, N_SLOTS, 2], mybir.dt.uint32, isOutput=True
    )

    # x*_r[r][p, c8, t] = x*[32p + 8r + c8, t]: 4KB contiguous DRAM runs
    xh_r = xh_d[:].rearrange("(p r c) t -> r p c t", p=P, r=NR)
    xl_r = xl_d[:].rearrange("(p r c) t -> r p c t", p=P, r=NR)

    with TileContext(nc) as tc:
        with (
            tc.tile_pool(name="xpool", bufs=xpool_bufs) as xpool,
            tc.tile_pool(name="cpool", bufs=1) as cpool,
            tc.tile_pool(name="spool", bufs=4) as spool,
            tc.tile_pool(name="psum", bufs=2, space="PSUM") as psum_pool,
        ):
            w_hl = cpool.tile([P, HC, EW], mybir.dt.bfloat16)
            nc.sync.dma_start(w_hl[:], whl_d[:])
            w_h0 = cpool.tile([P, HC, EW], mybir.dt.bfloat16)
            nc.sync.dma_start(w_h0[:], wh0_d[:])

            cpool.sorted_w = cpool.tile([P, N_SLOTS, E], mybir.dt.float32)
            cpool.idx_w = cpool.tile([P, N_SLOTS, E], mybir.dt.uint32)
            cpool.acc = cpool.tile([P, N_SLOTS, EW], mybir.dt.float32)

            pools = (xpool, cpool, spool, psum_pool)
            for _rep in range(reps):
                _emit_body(
                    nc, mybir, pools, w_hl, w_h0, xh_r, xl_r, wq, iq
                )
    if legalize:
        _legalize_sync_waits(nc, mybir)
    return nc


def _bf16_split(a):
    """Round-to-nearest bf16 hi/lo pair: a ~ hi + lo with ~2^-18 residual."""
    import ml_dtypes

    bf = ml_dtypes.bfloat16
    hi = a.astype(bf)
    lo = (a - hi.astype(np.float32)).astype(bf)
    return hi, lo


def pack_w(gate_weight):
    """w_hl[p, hc, 0:8]=bf16(W)[e, 32p+hc], [8:16]=bf16(W-Wh); w_h0 hi|0."""
    gw = np.ascontiguousarray(np.asarray(gate_weight, dtype=np.float32))
    wh, wl = _bf16_split(gw)  # [E, H] bf16 each
    w_hl = np.zeros((P, HC, EW), wh.dtype)
    w_h0 = np.zeros((P, HC, EW), wh.dtype)
    w_hl[:, :, 0:E] = wh.reshape(E, P, HC).transpose(1, 2, 0)
    w_hl[:, :, E:EW] = wl.reshape(E, P, HC).transpose(1, 2, 0)
    w_h0[:, :, 0:E] = w_hl[:, :, 0:E]
    return w_hl, w_h0


def shard_inputs(hidden_states, gate_weight):
    hs = np.asarray(hidden_states, dtype=np.float32).reshape(TOKENS_TOTAL, H)
    w_hl, w_h0 = pack_w(gate_weight)
    xT = np.ascontiguousarray(hs.T)  # [H, TOKENS_TOTAL]
    xh, xl = _bf16_split(xT)
    return [
        {
            "xh": np.ascontiguousarray(
                xh[:, c * TOKENS_PER_CORE : (c + 1) * TOKENS_PER_CORE]
            ),
            "xl": np.ascontiguousarray(
                xl[:, c * TOKENS_PER_CORE : (c + 1) * TOKENS_PER_CORE]
            ),
            "whl": w_hl,
            "wh0": w_h0,
        }
        for c in range(N_CORES)
    ]


def assemble(results):
    ws, idxs = [], []
    for c in range(N_CORES):
        wq = np.asarray(results[c]["wq"]).reshape(P, N_SLOTS, 2)
        iq = np.asarray(results[c]["iq"]).reshape(P, N_SLOTS, 2)
        # token (core-local) = slot*128 + p
        ws.append(np.transpose(wq, (1, 0, 2)).reshape(TOKENS_PER_CORE, 2))
        idxs.append(np.transpose(iq, (1, 0, 2)).reshape(TOKENS_PER_CORE, 2))
    w_full = np.concatenate(ws, 0).reshape(4, 4096, 2).astype(np.float32)
    i_full = np.concatenate(idxs, 0).reshape(4, 4096, 2).astype(np.int32)
    return w_full, i_full


BEST_CONFIG = {"xpool_bufs": 2}


def kernel(hidden_states, gate_weight):
    from concourse.bass_utils import run_bass_kernel_spmd

    nc = build_program(**BEST_CONFIG)
    in_maps = shard_inputs(hidden_states, gate_weight)
    br = run_bass_kernel_spmd(nc, in_maps, list(range(N_CORES)), trace=False)
    return assemble(br.results)
